# revision 35
# baseline (speedup 1.0000x reference)
"""Trainium2 Bass kernel for nn_AttentionLayer (attention pooling).

Reference math (per batch row b):
    u   = tanh(x[b] @ W + b_vec)        # [T, M]
    s   = u @ us                        # [T]
    a   = softmax(s) * mask / sum       # [T]  (mask is all ones per spec)
    out = a @ x[b]                      # [D]

Strategy: data-parallel over batch, B=32 rows -> 4 rows per NeuronCore on
8 cores.  The kernel is HBM-stream-bound (~85-100us to stream x at the
per-core share of HBM bandwidth), so every engine is kept under the
stream budget:
  - x is uploaded host-transposed per row as xh[q, p, c, tq] =
    x[t, 128c + p] (t = 512q + tq): each quarter DMA reads 16KB
    contiguous per partition and lands as x^T tiles [p=d, c, t] directly
    usable as the d-contracted matmul rhs -- no transposes on the hot
    path;
  - the f32->bf16 cast rides the DMA itself (SWDGE dtype-cast) so the
    DVE never touches the raw stream;
  - u^T = tanh(W^T x^T + b) in PSUM, tanh fused on ScalarE;
  - scores: us is replicated into all 128 PE columns, so one N=512
    matmul per quarter yields broadcast scores s[128, 512] in PSUM; exp
    on ScalarE writes broadcast e-rows plus the row-sum partials Z
    (accum_out);
  - pooling out[d] = sum_t e[t] x^T[d, t]: chunks 0..5 of each quarter
    run on the DVE as fused scalar_tensor_tensor (mult+mult,
    accum_out=sum) granules; chunks 6..7 run on the PE (transpose the
    x^T tiles back to natural, then e^T-column matmuls accumulating
    into a [P, 8] column PSUM) -- this both relieves the 1x-mode DVE
    and keeps the PE's HAM activity window alive between quarters.  The
    last row's final quarter runs ALL chunks on the PE so the
    post-stream tail is only a short exp->c2 chain;
  - one-slot software pipeline: each piece's exp-dependent PE block
    (e^T transpose + e-col copy + c2 matmuls) is deferred into the next
    piece's slot so the PE never stalls waiting for exp; row finishes
    ride the same deferral;
  - y is stored [p, c] per row and unshuffled on the host.
"""
import numpy as np

import concourse.bacc as bacc
import concourse.mybir as mybir
from concourse.tile import TileContext
from concourse.masks import make_identity
from concourse.bass_utils import run_bass_kernel_spmd

F32 = mybir.dt.float32
BF16 = mybir.dt.bfloat16

B, T, D, M = 32, 2048, 1024, 128
NCORES = 8
B_SH = B // NCORES   # 4 batch rows per core
P = 128
NCD = D // P         # 8 d-chunks
NQ = 4               # t-quarters per row
TQ = T // NQ         # 512 t per quarter
NTT = TQ // P        # 4 t-tiles per quarter
LAST = B_SH - 1
WARMUP = 40
NPE = 2              # chunks pooled on PE for normal quarters
NDVE = NCD - NPE


def _pieces_of(r, q):
    # last-row tail: final quarter split so the post-stream chain is short
    if r == LAST and q == NQ - 1:
        return [(0, 384), (384, 128)]
    return [(0, TQ)]


def _dma_pieces_of(r, q):
    # first DMA split small so descriptor-gen latency doesn't delay the
    # stream start; elsewhere 2MB quarters
    if r == 0 and q == 0:
        return [(0, 128), (128, 128), (256, 256)]
    return _pieces_of(r, q)


def _build_nc():
    nc = bacc.Bacc("TRN2", target_bir_lowering=False, debug=False,
                   num_devices=NCORES)
    # x host-rearranged: xh[r, q, p, c, tq] = x[r, 512q+tq, 128c+p]
    x = nc.declare_dram_parameter("x", [B_SH, NQ, P, NCD, TQ], F32,
                                  isOutput=False)
    # W host-rearranged to lhsT layout: W_r[p, c, m] = W[128c+p, m]
    W = nc.declare_dram_parameter("W", [P, NCD, M], F32, isOutput=False)
    b = nc.declare_dram_parameter("b", [1, M], F32, isOutput=False)
    us = nc.declare_dram_parameter("us", [1, M], F32, isOutput=False)
    # y[r, p, c] = out[r, 128c+p]
    y = nc.declare_dram_parameter("y", [B_SH, P, NCD], F32, isOutput=True)

    with TileContext(nc) as tc:
        with (
            tc.tile_pool(name="singles", bufs=1) as singles,
            tc.tile_pool(name="xb", bufs=3) as xb_pool,
            tc.tile_pool(name="esb", bufs=3) as e_pool,
            tc.tile_pool(name="usb", bufs=3) as u_pool,
            tc.tile_pool(name="xn", bufs=3) as xn_pool,
            tc.tile_pool(name="xn8", bufs=8) as xn8_pool,
            tc.tile_pool(name="ec", bufs=3) as ec_pool,
            tc.tile_pool(name="rowacc", bufs=16) as acc_pool,
            tc.tile_pool(name="scr", bufs=2) as scr_pool,
            tc.tile_pool(name="fin", bufs=2) as fin_pool,
            tc.tile_pool(name="up_ps", bufs=2, space="PSUM") as u_psum,
            tc.tile_pool(name="sb_ps", bufs=1, space="PSUM") as s_psum,
            tc.tile_pool(name="bc_ps", bufs=1, space="PSUM") as bc_psum,
            tc.tile_pool(name="tp_ps", bufs=3, space="PSUM") as tp_psum,
            tc.tile_pool(name="oc_ps", bufs=1, space="PSUM") as oc_psum,
        ):
            # constants on the sync HWDGE queue (separate from the x
            # stream's SWDGE queue)
            w_f32 = singles.tile([P, NCD, M], F32)
            nc.sync.dma_start(out=w_f32, in_=W[:, :, :])
            b_row = singles.tile([1, M], F32)
            nc.sync.dma_start(out=b_row, in_=b[:, :])
            us_row = singles.tile([1, M], F32)
            nc.sync.dma_start(out=us_row, in_=us[:, :])

            # x stream: SWDGE cast-DMA straight into bf16 row buffers
            xb_tiles = {}

            def emit_row_dmas(r):
                xb_r = xb_pool.tile([P, NQ, NCD, TQ], BF16, tag="xb",
                                    name=f"xb_{r}")
                xb_tiles[r] = xb_r
                for q in range(NQ):
                    for (t0, tw) in _dma_pieces_of(r, q):
                        nc.gpsimd.dma_start(
                            out=xb_r[:, q, :, t0:t0 + tw],
                            in_=x[r, q][:, :, t0:t0 + tw],
                        )

            emit_row_dmas(0)

            # init constants on DVE
            one_f32 = singles.tile([1, 1], F32)
            nc.vector.memset(one_f32, 1.0)
            ones_bf = singles.tile([P, P], BF16)
            nc.vector.memset(ones_bf, 1.0)
            ident = singles.tile([P, P], BF16)
            make_identity(nc, ident)
            w_bf = singles.tile([P, NCD, M], BF16)
            nc.vector.tensor_copy(out=w_bf, in_=w_f32)

            emit_row_dmas(1)
            emit_row_dmas(2)

            # PE warm-up while the first quarters stream in (into the
            # up-tag ring so no extra PSUM bank is spent)
            warm = u_psum.tile([P, TQ], F32, tag="up", name="warm")
            for i in range(WARMUP):
                nc.tensor.matmul(warm[:, 0:P], ones_bf, ones_bf,
                                 start=True, stop=True)

            # b/us -> per-partition layout via K=1 matmuls
            bc = bc_psum.tile([P, 2], F32, tag="bc")
            nc.tensor.matmul(bc[:, 0:1], b_row, one_f32, start=True, stop=True)
            nc.tensor.matmul(bc[:, 1:2], us_row, one_f32, start=True, stop=True)
            b_sb = singles.tile([P, 1], F32)
            nc.vector.tensor_copy(out=b_sb, in_=bc[:, 0:1])
            us_sc = singles.tile([P, 1], F32)
            nc.vector.tensor_copy(out=us_sc, in_=bc[:, 1:2])
            # us replicated into 128 identical PE columns
            us_bc = singles.tile([P, P], BF16)
            nc.vector.tensor_scalar_mul(us_bc, ones_bf, us_sc)

            # one-slot software pipeline (see module docstring)
            deferred = [None]
            pending_fin = [None]

            def drain():
                if deferred[0] is not None:
                    f, deferred[0] = deferred[0], None
                    f()
                if pending_fin[0] is not None:
                    f, pending_fin[0] = pending_fin[0], None
                    f()

            for r in range(B_SH):
                xb_r = xb_tiles[r]
                e_sb = e_pool.tile([P, NQ, TQ], BF16, tag="e", name=f"e_{r}")
                rs = acc_pool.tile([P, 8], F32, tag="rs", name=f"rs_{r}")
                oc = oc_psum.tile([P, NCD], F32, tag="oc", name=f"oc_{r}")
                acc_list = []
                run_tot = [None]
                n_rs = 0
                oc_i = [0]
                n_oc = 2 * NQ * NTT if r < LAST else (
                    2 * (NQ - 1) * NTT + NCD * NTT)

                if r + 3 < B_SH:
                    emit_row_dmas(r + 3)

                def emit_pool_dve(granules, r=r, xb_r=xb_r, e_sb=e_sb,
                                  acc_list=acc_list, run_tot=run_tot):
                    """fused (x*e, accum=sum) on DVE, chunks 0..NDVE-1;
                    partials merge into a running total as they land."""
                    for (q0, q1, t0, tw) in granules:
                        dst = acc_pool.tile([P, NDVE], F32, tag="acc",
                                            name=f"acc_{r}_{len(acc_list)}")
                        acc_list.append(dst)
                        scr = scr_pool.tile([P, q1 - q0, tw], BF16,
                                            tag=f"scr{q1 - q0}_{tw}",
                                            name=f"scr_{r}_{len(acc_list)}")
                        for c in range(NDVE):
                            nc.vector.scalar_tensor_tensor(
                                out=scr,
                                in0=xb_r[:, q0:q1, c, t0:t0 + tw],
                                scalar=1.0,
                                in1=e_sb[:, q0:q1, t0:t0 + tw],
                                op0=mybir.AluOpType.mult,
                                op1=mybir.AluOpType.mult,
                                accum_out=dst[:, c:c + 1],
                            )
                        if run_tot[0] is None:
                            run_tot[0] = dst
                        else:
                            nxt = fin_pool.tile(
                                [P, NDVE], F32, tag="tot",
                                name=f"tot_{r}_{len(acc_list)}")
                            nc.vector.tensor_tensor(
                                out=nxt, in0=run_tot[0], in1=dst,
                                op=mybir.AluOpType.add)
                            run_tot[0] = nxt

                def c2_col(c, xn_ap, ecol_ap, oc=oc, oc_i=oc_i, n_oc=n_oc):
                    # single start/stop for the whole per-row group: start
                    # clears has_written for the ENTIRE bank
                    nc.tensor.matmul(
                        oc[:, c:c + 1], xn_ap, ecol_ap,
                        start=(oc_i[0] == 0), stop=(oc_i[0] == n_oc - 1))
                    oc_i[0] += 1

                for q in range(NQ):
                    full_pe = (r == LAST and q == NQ - 1)
                    ecq = ec_pool.tile([P, NTT], BF16, tag="ec",
                                       name=f"ec_{r}_{q}")
                    if not full_pe:
                        xn67 = xn_pool.tile([P, NPE, NTT, P], BF16,
                                            tag="xn67")
                    for (t0, tw) in _pieces_of(r, q):
                        up = u_psum.tile([P, TQ], F32, tag="up")
                        for c in range(NCD):
                            nc.tensor.matmul(
                                up[:, t0:t0 + tw],
                                w_bf[:, c, :],
                                xb_r[:, q, c, t0:t0 + tw],
                                start=(c == 0), stop=(c == NCD - 1),
                            )
                        # x^T -> natural transposes for the PE-pooled chunks;
                        # only xb-dependent, so they run under the stream
                        xn_piece = {}
                        if full_pe:
                            for j in range(t0 // P, (t0 + tw) // P):
                                tpa = tp_psum.tile([P, 4 * P], BF16, tag="tp")
                                tpb = tp_psum.tile([P, 4 * P], BF16, tag="tp")
                                for c in range(4):
                                    nc.tensor.transpose(
                                        tpa[:, c * P:(c + 1) * P],
                                        xb_r[:, q, c, j * P:(j + 1) * P],
                                        ident)
                                    nc.tensor.transpose(
                                        tpb[:, c * P:(c + 1) * P],
                                        xb_r[:, q, 4 + c, j * P:(j + 1) * P],
                                        ident)
                                xn = xn8_pool.tile([P, NCD, P], BF16,
                                                   tag="xn8")
                                xn_piece[j] = xn
                                nc.scalar.copy(
                                    out=xn[:, 0:4, :],
                                    in_=tpa.rearrange("p (c t) -> p c t",
                                                      c=4))
                                nc.scalar.copy(
                                    out=xn[:, 4:8, :],
                                    in_=tpb.rearrange("p (c t) -> p c t",
                                                      c=4))
                        elif t0 == 0:
                            tpx = tp_psum.tile([P, 4 * P], BF16, tag="tp")
                            tpy = tp_psum.tile([P, 4 * P], BF16, tag="tp")
                            for j in range(NTT):
                                nc.tensor.transpose(
                                    tpx[:, j * P:(j + 1) * P],
                                    xb_r[:, q, NDVE, j * P:(j + 1) * P],
                                    ident)
                                nc.tensor.transpose(
                                    tpy[:, j * P:(j + 1) * P],
                                    xb_r[:, q, NDVE + 1, j * P:(j + 1) * P],
                                    ident)
                            nc.scalar.copy(
                                out=xn67[:, 0, :, :],
                                in_=tpx.rearrange("p (j t) -> p j t", j=NTT))
                            nc.scalar.copy(
                                out=xn67[:, 1, :, :],
                                in_=tpy.rearrange("p (j t) -> p j t", j=NTT))
                        drain()
                        u_sb = u_pool.tile([P, TQ], BF16, tag="u")
                        nc.scalar.activation(
                            out=u_sb[:, t0:t0 + tw], in_=up[:, t0:t0 + tw],
                            func=mybir.ActivationFunctionType.Tanh,
                            bias=b_sb, scale=1.0,
                        )
                        sb = s_psum.tile([P, TQ], F32, tag="sb")
                        nc.tensor.matmul(
                            sb[:, t0:t0 + tw], us_bc, u_sb[:, t0:t0 + tw],
                            start=True, stop=True,
                        )
                        nc.scalar.activation(
                            out=e_sb[:, q, t0:t0 + tw], in_=sb[:, t0:t0 + tw],
                            func=mybir.ActivationFunctionType.Exp,
                            accum_out=rs[:, n_rs:n_rs + 1],
                        )
                        n_rs += 1

                        def pool_pe(q=q, t0=t0, tw=tw, e_sb=e_sb, ecq=ecq,
                                    xn_piece=xn_piece, full_pe=full_pe,
                                    xn67=(None if full_pe else xn67),
                                    c2_col=c2_col):
                            # e^T columns via PE transpose of the broadcast
                            # e-rows (any output column equals e^T), then the
                            # c2 column-matmuls
                            j0, j1 = t0 // P, (t0 + tw) // P
                            tpe = tp_psum.tile([P, 4 * P], BF16, tag="tp")
                            for j in range(j0, j1):
                                nc.tensor.transpose(
                                    tpe[:, (j - j0) * P:(j - j0 + 1) * P],
                                    e_sb[:, q, j * P:(j + 1) * P], ident)
                            nc.scalar.copy(
                                out=ecq[:, j0:j1],
                                in_=tpe.rearrange("p (j t) -> p j t",
                                                  j=4)[:, 0:j1 - j0, 0])
                            if full_pe:
                                for j in range(j0, j1):
                                    xn = xn_piece[j]
                                    for c in range(NCD):
                                        c2_col(c, xn[:, c, :],
                                               ecq[:, j:j + 1])
                            else:
                                for j in range(j0, j1):
                                    c2_col(NDVE, xn67[:, 0, j, :],
                                           ecq[:, j:j + 1])
                                    c2_col(NDVE + 1, xn67[:, 1, j, :],
                                           ecq[:, j:j + 1])

                        deferred[0] = pool_pe
                    # DVE pooling emission (chunks 0..NDVE-1): rows 0..2
                    # use half-row granules (halves the instruction count --
                    # each STT pays a ~0.5us accumulator-drain turnaround);
                    # the last row chases per quarter
                    if r == LAST:
                        # the last row chases the stream per quarter: the
                        # earlier dependency (exp of each quarter) beats the
                        # drain-turnaround savings of coarser granules here
                        if not full_pe:
                            emit_pool_dve([(q, q + 1, 0, TQ)])
                    elif q in (1, 3):
                        emit_pool_dve([(q - 1, q + 1, 0, TQ)])

                def finish(r=r, rs=rs, n_rs=n_rs, run_tot=run_tot, oc=oc):
                    # Z, 1/Z, scale, store (partials already merged)
                    zr = fin_pool.tile([P, 1], F32, tag="z")
                    nc.vector.tensor_reduce(
                        out=zr, in_=rs[:, 0:n_rs],
                        axis=mybir.AxisListType.X, op=mybir.AluOpType.add)
                    inv = fin_pool.tile([P, 1], F32, tag="inv")
                    nc.vector.reciprocal(out=inv, in_=zr)
                    tot = run_tot[0]
                    if r == LAST:
                        # the full-PE final quarter contributed chunks 0..5
                        # into oc as well
                        nxt = fin_pool.tile([P, NDVE], F32, tag="tot",
                                            name=f"tot_{r}_oc")
                        nc.vector.tensor_tensor(out=nxt, in0=tot,
                                                in1=oc[:, 0:NDVE],
                                                op=mybir.AluOpType.add)
                        tot = nxt
                    y_sb = fin_pool.tile([P, NCD], F32, tag="y")
                    nc.vector.tensor_scalar_mul(y_sb[:, 0:NDVE], tot, inv)
                    nc.vector.tensor_scalar_mul(y_sb[:, NDVE:NCD],
                                                oc[:, NDVE:NCD], inv)
                    nc.sync.dma_start(out=y[r], in_=y_sb)

                pending_fin[0] = finish

            # flush the pipeline: last piece's pooling block + last finish
            drain()

    nc.compile()
    return nc


_NC_CACHE = []


def _numpy_reference(x, W, b, us, mask):
    m = mask.astype(x.dtype)
    u = np.tanh(np.einsum('btd,dm->btm', x, W) + b)
    utu = np.einsum('btm,mo->bto', u, us)[..., 0]
    e = np.exp(utu - utu.max(axis=-1, keepdims=True))
    e = m * e
    a = e / e.sum(axis=-1, keepdims=True)
    return np.einsum('bt,btd->bd', a, x).astype(np.float32)


def make_in_maps(x, W, b, us):
    """Per-core input dicts; x/W host-rearranged (layout only, dtypes kept)."""
    x = np.ascontiguousarray(np.asarray(x, dtype=np.float32))
    W = np.ascontiguousarray(np.asarray(W, dtype=np.float32))
    b = np.ascontiguousarray(np.asarray(b, dtype=np.float32))
    us = np.ascontiguousarray(np.asarray(us, dtype=np.float32))
    W_r = np.ascontiguousarray(W.reshape(NCD, P, M).transpose(1, 0, 2))
    b_r = np.ascontiguousarray(b.reshape(1, M))
    us_r = np.ascontiguousarray(us.reshape(M, 1).T)
    maps = []
    for i in range(NCORES):
        xc = x[i * B_SH:(i + 1) * B_SH]                # [B_SH, T, D]
        xh = xc.reshape(B_SH, NQ, TQ, NCD, P).transpose(0, 1, 4, 3, 2)
        maps.append({
            "x": np.ascontiguousarray(xh),
            "W": W_r, "b": b_r, "us": us_r,
        })
    return maps


def gather_outputs(res):
    outs = []
    for i in range(NCORES):
        yr = res.results[i]["y"]               # [B_SH, P, NCD]
        outs.append(yr.transpose(0, 2, 1).reshape(B_SH, D))
    return np.ascontiguousarray(np.concatenate(outs, axis=0))


def kernel(x, W, b, us, mask):
    x = np.ascontiguousarray(np.asarray(x, dtype=np.float32))
    W = np.ascontiguousarray(np.asarray(W, dtype=np.float32))
    b = np.ascontiguousarray(np.asarray(b, dtype=np.float32))
    us = np.ascontiguousarray(np.asarray(us, dtype=np.float32))
    mask = np.asarray(mask)

    if not bool(mask.all()):
        # spec guarantees an all-ones mask; exact numpy fallback otherwise
        return _numpy_reference(x, W, b, us, mask)

    if not _NC_CACHE:
        _NC_CACHE.append(_build_nc())
    nc = _NC_CACHE[0]

    in_maps = make_in_maps(x, W, b, us)
    res = run_bass_kernel_spmd(nc, in_maps, core_ids=list(range(NCORES)),
                               trace=False)
    return gather_outputs(res)
